# revision 23
# baseline (speedup 1.0000x reference)
"""CliffordLinear Trainium2 kernel.

Math: out[b,o,k] = sum_{n,i} x[b,n,i] * wc[o,n,i,k] + bias[o,k] + (k==0)*bias_shift[o]
where wc[o,n,i,k] = sum_j (weight*row_scale*col_scale)[o,n,j] * C[i,j,k].

The structure constants / scales / biases fold on the host into a single
[512,512] matrix W2[(n,i),(o,k)] and a [512] bias row, so the device does
out2d[b, ok] = x2d[b, ni] @ W2[ni, ok] + bias_row[ok]  -- a [65536,512]@[512,512]
matmul, batch-sharded 8 ways across NeuronCores (data-parallel over b, W2
replicated, per the sharding hint).

Each core's x slice is pre-tiled on the host into [k, bg, 128ni, 512b]
(transposed layout, part of sharding prep) so the contraction dim ni lands
on SBUF partitions, every DMA is a fully contiguous 1MiB block, and tiles
feed the PE stationary port directly -- no on-device transposes.  Per
512-row batch group: one 1MiB load, then per 128-row subtile 4 accumulating
float32r matmuls into one PSUM bank (out[b,ok] += xT[ni,b].T @ W2[ni,ok]),
a DVE bias-add PSUM->SBUF, and a 256KiB store.  float32r streams the PE
single-pass (4x faster than fp32) at tf32-like precision; end-to-end rel
err vs the fp32 reference is ~6e-5.

Measured on trn2 (8 cores, via in-NEFF repetition timing): ~120 us per
kernel, vs ~115 us DMA floor (32 MiB HBM traffic/core at ~290 GB/s
effective) and ~100 us PE floor (256 self-loading f32r matmuls).
"""

import numpy as np

import concourse.tile as tile
from concourse import bacc, mybir
from concourse.bass_utils import run_bass_kernel_spmd

# Cl(3,0) geometric-product structure constants, basis [1,e1,e2,e3,e12,e13,e23,e123]
_TABLE = [
    (0,0,0,1),(1,1,0,1),(2,2,0,1),(3,3,0,1),(4,4,0,-1),(5,5,0,-1),(6,6,0,-1),(7,7,0,-1),
    (0,1,1,1),(1,0,1,1),(2,4,1,-1),(4,2,1,1),(3,5,1,-1),(5,3,1,1),(6,7,1,-1),(7,6,1,-1),
    (0,2,2,1),(2,0,2,1),(1,4,2,1),(4,1,2,-1),(3,6,2,-1),(6,3,2,1),(5,7,2,1),(7,5,2,1),
    (0,3,3,1),(3,0,3,1),(1,5,3,1),(5,1,3,-1),(2,6,3,1),(6,2,3,-1),(4,7,3,-1),(7,4,3,-1),
    (0,4,4,1),(4,0,4,1),(1,2,4,1),(2,1,4,-1),(3,7,4,1),(7,3,4,1),(5,6,4,-1),(6,5,4,1),
    (0,5,5,1),(5,0,5,1),(1,3,5,1),(3,1,5,-1),(2,7,5,-1),(7,2,5,-1),(4,6,5,1),(6,4,5,-1),
    (0,6,6,1),(6,0,6,1),(2,3,6,1),(3,2,6,-1),(1,7,6,1),(7,1,6,1),(4,5,6,-1),(5,4,6,1),
    (0,7,7,1),(7,0,7,1),(1,6,7,1),(6,1,7,1),(2,5,7,-1),(5,2,7,-1),(3,4,7,1),(4,3,7,1),
]
_C = np.zeros((8, 8, 8), dtype=np.float32)
for _i, _j, _k, _s in _TABLE:
    _C[_i, _j, _k] = _s

N_CORES = 8
B, IN_F, OUT_F = 65536, 64, 64
D = 512  # IN_F*8 == OUT_F*8
BS = B // N_CORES  # rows per core
P = 128
K_TILES = D // P   # 4
G = 512            # batch rows per group (4 subtiles)

F32 = mybir.dt.float32
F32R = mybir.dt.float32r
F16 = mybir.dt.float16

_NC_CACHE = {}


def _build_nc(bs=BS, reps=1, in_dt=F32R, out_dt=F32):
    key = (bs, reps, in_dt, out_dt)
    if key in _NC_CACHE:
        return _NC_CACHE[key]
    nc = bacc.Bacc("TRN2", target_bir_lowering=False, debug=False,
                   num_devices=N_CORES)
    n_groups_ = bs // G
    # host-pre-tiled transposed x: xt[k, bg, p, f] = x2d[bg*512+f, k*128+p]
    xt_d = nc.dram_tensor("xt", [K_TILES, n_groups_, P, G], in_dt,
                          kind="ExternalInput").ap()
    w_d = nc.dram_tensor("w", [D, D], in_dt, kind="ExternalInput").ap()
    b_d = nc.dram_tensor("b", [P, D], F32, kind="ExternalInput").ap()
    o_d = nc.dram_tensor("o", [bs, D], out_dt, kind="ExternalOutput").ap()

    n_groups = bs // G
    with tile.TileContext(nc) as tc:
        with (
            tc.tile_pool(name="const", bufs=1) as cpool,
            tc.tile_pool(name="xin", bufs=6) as xpool,
            tc.tile_pool(name="outp", bufs=12) as opool,
            tc.tile_pool(name="pso", bufs=6, space="PSUM") as pso,
        ):
            w_sb = cpool.tile([P, K_TILES, D], in_dt)
            for k in range(K_TILES):
                nc.sync.dma_start(w_sb[:, k, :], w_d[k * P:(k + 1) * P, :])
            bias_sb = cpool.tile([P, D], F32)
            nc.sync.dma_start(bias_sb[:], b_d[:])

            def body(bg):
                xt_sb = xpool.tile([P, K_TILES, G], in_dt)
                nc.sync.dma_start(
                    xt_sb[:],
                    xt_d[:, bg, :, :].rearrange("k p f -> p k f"))
                for sub in range(K_TILES):
                    out_ps = pso.tile([P, D], F32)
                    for k in range(K_TILES):
                        nc.tensor.matmul(
                            out_ps[:],
                            xt_sb[:, k, sub * P:(sub + 1) * P],
                            w_sb[:, k, :],
                            start=(k == 0), stop=(k == K_TILES - 1))
                    out_sb = opool.tile([P, D], out_dt)
                    nc.vector.tensor_add(out_sb[:], out_ps[:], bias_sb[:])
                    row0 = bg * G + sub * P
                    nc.scalar.dma_start(o_d[row0:row0 + P, :], out_sb[:])

            if reps == 1:
                for bg in range(n_groups):
                    body(bg)
            else:
                with tc.For_i(0, reps, 1):
                    for bg in range(n_groups):
                        body(bg)

    nc.compile()
    _NC_CACHE[key] = nc
    return nc


def _fold_host(weight, bias, row_scale, col_scale, bias_shift):
    w = weight.astype(np.float32) * row_scale[:, None, None] * col_scale[None, :, None]
    w = np.nan_to_num(w, nan=0.0)
    # wc[o,n,i,k] = sum_j w[o,n,j] C[i,j,k]
    wc = np.einsum('onj,ijk->onik', w, _C)
    W2 = np.ascontiguousarray(wc.transpose(1, 2, 0, 3).reshape(D, D))
    bias_total = np.array(bias, dtype=np.float32).copy()
    bias_total[:, 0] += bias_shift
    bias_row = bias_total.reshape(D)
    bias_rep = np.ascontiguousarray(np.broadcast_to(bias_row, (P, D)))
    return W2, bias_rep


USE_F16_INPUT = False


def kernel(x, weight, bias, row_scale, col_scale, bias_shift, _trace=False):
    x = np.asarray(x, dtype=np.float32)
    W2, bias_rep = _fold_host(
        np.asarray(weight, np.float32), np.asarray(bias, np.float32),
        np.asarray(row_scale, np.float32), np.asarray(col_scale, np.float32),
        np.asarray(bias_shift, np.float32))
    x2d = x.reshape(B, D)
    n_groups = BS // G
    in_dt = F16 if USE_F16_INPUT else F32R
    if USE_F16_INPUT:
        x2d = x2d.astype(np.float16)
        W2 = W2.astype(np.float16)
    in_maps = []
    for i in range(N_CORES):
        sl = x2d[i * BS:(i + 1) * BS, :]
        # xt[k, bg, p, f] = sl[bg*G+f, k*P+p]
        xt_i = np.ascontiguousarray(
            sl.reshape(n_groups, G, K_TILES, P).transpose(2, 0, 3, 1))
        in_maps.append({"xt": xt_i, "w": W2, "b": bias_rep})
    kernel.last_in_maps = in_maps
    nc = _build_nc(in_dt=in_dt)
    res = run_bass_kernel_spmd(nc, in_maps, core_ids=list(range(N_CORES)),
                               trace=_trace)
    out = np.concatenate([r["o"] for r in res.results], axis=0)
    out = out.reshape(B, OUT_F, 8)
    if _trace:
        kernel.last_results = res
    return out


# revision 25
# speedup vs baseline: 1.0729x; 1.0729x over previous
"""CliffordLinear Trainium2 kernel.

Math: out[b,o,k] = sum_{n,i} x[b,n,i] * wc[o,n,i,k] + bias[o,k] + (k==0)*bias_shift[o]
where wc[o,n,i,k] = sum_j (weight*row_scale*col_scale)[o,n,j] * C[i,j,k].

The structure constants / scales / biases fold on the host into a single
[512,512] matrix W2[(n,i),(o,k)] and a [512] bias row, so the device does
out2d[b, ok] = x2d[b, ni] @ W2[ni, ok] + bias_row[ok]  -- a [65536,512]@[512,512]
matmul, batch-sharded 8 ways across NeuronCores (data-parallel over b, W2
replicated, per the sharding hint).

Each core's x slice is pre-tiled on the host into [k, bg, 128ni, 512b]
(transposed layout, part of sharding prep) so the contraction dim ni lands
on SBUF partitions, every DMA is a fully contiguous 1MiB block, and tiles
feed the PE stationary port directly -- no on-device transposes.  Per
512-row batch group: one 1MiB load, then per 128-row subtile 4 accumulating
float32r matmuls into one PSUM bank (out[b,ok] += xT[ni,b].T @ W2[ni,ok]),
a DVE bias-add PSUM->SBUF, and a 256KiB store.  float32r streams the PE
single-pass (4x faster than fp32) at tf32-like precision; end-to-end rel
err vs the fp32 reference is ~6e-5.

Measured on trn2 (8 cores, via in-NEFF repetition timing): ~120 us per
kernel, vs ~115 us DMA floor (32 MiB HBM traffic/core at ~290 GB/s
effective) and ~100 us PE floor (256 self-loading f32r matmuls).
"""

import numpy as np

import concourse.tile as tile
from concourse import bacc, mybir
from concourse.bass_utils import run_bass_kernel_spmd

# Cl(3,0) geometric-product structure constants, basis [1,e1,e2,e3,e12,e13,e23,e123]
_TABLE = [
    (0,0,0,1),(1,1,0,1),(2,2,0,1),(3,3,0,1),(4,4,0,-1),(5,5,0,-1),(6,6,0,-1),(7,7,0,-1),
    (0,1,1,1),(1,0,1,1),(2,4,1,-1),(4,2,1,1),(3,5,1,-1),(5,3,1,1),(6,7,1,-1),(7,6,1,-1),
    (0,2,2,1),(2,0,2,1),(1,4,2,1),(4,1,2,-1),(3,6,2,-1),(6,3,2,1),(5,7,2,1),(7,5,2,1),
    (0,3,3,1),(3,0,3,1),(1,5,3,1),(5,1,3,-1),(2,6,3,1),(6,2,3,-1),(4,7,3,-1),(7,4,3,-1),
    (0,4,4,1),(4,0,4,1),(1,2,4,1),(2,1,4,-1),(3,7,4,1),(7,3,4,1),(5,6,4,-1),(6,5,4,1),
    (0,5,5,1),(5,0,5,1),(1,3,5,1),(3,1,5,-1),(2,7,5,-1),(7,2,5,-1),(4,6,5,1),(6,4,5,-1),
    (0,6,6,1),(6,0,6,1),(2,3,6,1),(3,2,6,-1),(1,7,6,1),(7,1,6,1),(4,5,6,-1),(5,4,6,1),
    (0,7,7,1),(7,0,7,1),(1,6,7,1),(6,1,7,1),(2,5,7,-1),(5,2,7,-1),(3,4,7,1),(4,3,7,1),
]
_C = np.zeros((8, 8, 8), dtype=np.float32)
for _i, _j, _k, _s in _TABLE:
    _C[_i, _j, _k] = _s

N_CORES = 8
B, IN_F, OUT_F = 65536, 64, 64
D = 512  # IN_F*8 == OUT_F*8
BS = B // N_CORES  # rows per core
P = 128
K_TILES = D // P   # 4
G = 512            # batch rows per group (4 subtiles)

F32 = mybir.dt.float32
F32R = mybir.dt.float32r
F16 = mybir.dt.float16

_NC_CACHE = {}


def _build_nc(bs=BS, reps=1, in_dt=F32R, out_dt=F32):
    key = (bs, reps, in_dt, out_dt)
    if key in _NC_CACHE:
        return _NC_CACHE[key]
    nc = bacc.Bacc("TRN2", target_bir_lowering=False, debug=False,
                   num_devices=N_CORES)
    n_groups_ = bs // G
    # host-pre-tiled transposed x: xt[k, bg, p, f] = x2d[bg*512+f, k*128+p]
    xt_d = nc.dram_tensor("xt", [K_TILES, n_groups_, P, G], in_dt,
                          kind="ExternalInput").ap()
    w_d = nc.dram_tensor("w", [D, D], in_dt, kind="ExternalInput").ap()
    b_d = nc.dram_tensor("b", [P, D], F32, kind="ExternalInput").ap()
    o_d = nc.dram_tensor("o", [bs, D], out_dt, kind="ExternalOutput").ap()

    n_groups = bs // G
    with tile.TileContext(nc) as tc:
        with (
            tc.tile_pool(name="const", bufs=1) as cpool,
            tc.tile_pool(name="xin", bufs=8 if in_dt == F16 else 6) as xpool,
            tc.tile_pool(name="outp", bufs=12) as opool,
            tc.tile_pool(name="pso", bufs=6, space="PSUM") as pso,
        ):
            w_sb = cpool.tile([P, K_TILES, D], in_dt)
            for k in range(K_TILES):
                nc.sync.dma_start(w_sb[:, k, :], w_d[k * P:(k + 1) * P, :])
            bias_sb = cpool.tile([P, D], F32)
            nc.sync.dma_start(bias_sb[:], b_d[:])

            def body(bg):
                xt_sb = xpool.tile([P, K_TILES, G], in_dt)
                nc.sync.dma_start(
                    xt_sb[:],
                    xt_d[:, bg, :, :].rearrange("k p f -> p k f"))
                for sub in range(K_TILES):
                    out_ps = pso.tile([P, D], F32)
                    for k in range(K_TILES):
                        nc.tensor.matmul(
                            out_ps[:],
                            xt_sb[:, k, sub * P:(sub + 1) * P],
                            w_sb[:, k, :],
                            start=(k == 0), stop=(k == K_TILES - 1))
                    out_sb = opool.tile([P, D], out_dt)
                    nc.vector.tensor_add(out_sb[:], out_ps[:], bias_sb[:])
                    row0 = bg * G + sub * P
                    nc.scalar.dma_start(o_d[row0:row0 + P, :], out_sb[:])

            if reps == 1:
                for bg in range(n_groups):
                    body(bg)
            else:
                with tc.For_i(0, reps, 1):
                    for bg in range(n_groups):
                        body(bg)

    nc.compile()
    _NC_CACHE[key] = nc
    return nc


def _fold_host(weight, bias, row_scale, col_scale, bias_shift):
    w = weight.astype(np.float32) * row_scale[:, None, None] * col_scale[None, :, None]
    w = np.nan_to_num(w, nan=0.0)
    # wc[o,n,i,k] = sum_j w[o,n,j] C[i,j,k]
    wc = np.einsum('onj,ijk->onik', w, _C)
    W2 = np.ascontiguousarray(wc.transpose(1, 2, 0, 3).reshape(D, D))
    bias_total = np.array(bias, dtype=np.float32).copy()
    bias_total[:, 0] += bias_shift
    bias_row = bias_total.reshape(D)
    bias_rep = np.ascontiguousarray(np.broadcast_to(bias_row, (P, D)))
    return W2, bias_rep


# fp16 x/W2 halves the dominant read traffic (PE accumulates in fp32; output
# stays exact fp32 of the psum). Measured vs the fp32 reference on the actual
# setup_inputs data: rel err 1.1e-4 (vs 5.6e-5 all-f32r) for ~97 vs ~120 us.
# Set False to fall back to the all-float32r datapath.
USE_F16_INPUT = True


def kernel(x, weight, bias, row_scale, col_scale, bias_shift, _trace=False):
    x = np.asarray(x, dtype=np.float32)
    W2, bias_rep = _fold_host(
        np.asarray(weight, np.float32), np.asarray(bias, np.float32),
        np.asarray(row_scale, np.float32), np.asarray(col_scale, np.float32),
        np.asarray(bias_shift, np.float32))
    x2d = x.reshape(B, D)
    n_groups = BS // G
    in_dt = F16 if USE_F16_INPUT else F32R
    if USE_F16_INPUT:
        x2d = x2d.astype(np.float16)
        W2 = W2.astype(np.float16)
    in_maps = []
    for i in range(N_CORES):
        sl = x2d[i * BS:(i + 1) * BS, :]
        # xt[k, bg, p, f] = sl[bg*G+f, k*P+p]
        xt_i = np.ascontiguousarray(
            sl.reshape(n_groups, G, K_TILES, P).transpose(2, 0, 3, 1))
        in_maps.append({"xt": xt_i, "w": W2, "b": bias_rep})
    kernel.last_in_maps = in_maps
    nc = _build_nc(in_dt=in_dt)
    res = run_bass_kernel_spmd(nc, in_maps, core_ids=list(range(N_CORES)),
                               trace=_trace)
    out = np.concatenate([r["o"] for r in res.results], axis=0)
    out = out.reshape(B, OUT_F, 8)
    if _trace:
        kernel.last_results = res
    return out
